# revision 16
# baseline (speedup 1.0000x reference)
"""Causal self-attention (B=2, T=2048, C=1024, H=16) on 8 trn2 NeuronCores.

Sharding: tensor-parallel over heads — 2 heads per core. Each core computes
q/k/v projections for its 2 heads (from a replicated transposed x), causal
attention for those heads, and a partial out-projection [B*T, C]; the host
sums the 8 partials and adds the output bias.

Layouts (per core):
  xT    [1024, 4096]  x transposed (c on partitions), host-prepared, bf16
  qT/kT [128, 4096]   head-dim-major (2 heads x 64 dims on partitions)
  v     natural [t, d] per head via PE transpose of vT
  S^T   [k, q] tiles from matmul(lhsT=kT, rhs=qT); softmax runs without the
        max-subtraction (scores are O(few)), the sum over k rides as a
        ones-column in the PV matmul, normalization divides at the end.

Perf structure:
  - the attention stage loop is paced by the ACT exp (2 x [128,2,512] exps
    per 2-k-tile stage); all projection / out-projection / v-transpose PE
    work is chopped into small closures and pumped through the stage loop so
    the PE fills the exp-wait gaps with useful work instead of idling.
  - out-projection PSUM lives in the proj pool (two 1-bank halves), so the
    scores pool is never stolen by fillers.
  - diagonal tiles: one full-width exp + a 128-col strip mask multiply.
  - all input DMAs are issued up front in need-order, split across 3 queues.
  - windows are emitted so the last attention is the small (b=1,w=0) one,
    keeping the end-of-kernel normalize/out-proj chain short.
"""

import sys

for _p in ("/opt/trn_rl_repo", "/opt/pypackages"):
    if _p not in sys.path:
        sys.path.append(_p)

from contextlib import ExitStack

import numpy as np
import ml_dtypes

import concourse.bass as bass
import concourse.tile as tile
from concourse import bacc, mybir
from concourse.bass import ts, ds
from concourse.bass_utils import run_bass_kernel_spmd
from concourse.masks import make_identity

BF16 = ml_dtypes.bfloat16
F32 = mybir.dt.float32
MBF16 = mybir.dt.bfloat16
AF = mybir.ActivationFunctionType

B, T, C, H = 2, 2048, 1024, 16
HD = C // H              # 64
NCORES = 8
HPC = H // NCORES        # 2 heads per core
BT = B * T               # 4096
SCALE = 1.0 / np.sqrt(HD)
NCO = C // 128           # 8 contraction tiles
NPW = BT // 512          # 8 projection windows
NQW = T // 512           # 4 q-windows per batch
NKT = T // 128           # 16 k-tiles per batch
NTT = BT // 128          # 32 t-tiles


def build_nc(dbg=False):
    nc = bacc.Bacc("TRN2", target_bir_lowering=False, debug=False)

    xT_d = nc.dram_tensor("xT", [NCO, 128, BT], MBF16, kind="ExternalInput").ap()
    wq_d = nc.dram_tensor("wq", [128, NCO, 128], MBF16, kind="ExternalInput").ap()
    wk_d = nc.dram_tensor("wk", [128, NCO, 128], MBF16, kind="ExternalInput").ap()
    wv_d = nc.dram_tensor("wv", [128, NCO, 128], MBF16, kind="ExternalInput").ap()
    bq_d = nc.dram_tensor("bq", [128, 1], F32, kind="ExternalInput").ap()
    bk_d = nc.dram_tensor("bk", [128, 1], F32, kind="ExternalInput").ap()
    bv_d = nc.dram_tensor("bv", [128, 1], F32, kind="ExternalInput").ap()
    wo_d = nc.dram_tensor("wo", [128, C], MBF16, kind="ExternalInput").ap()
    mk_d = nc.dram_tensor("mask", [128, 128], MBF16, kind="ExternalInput").ap()
    out_d = nc.dram_tensor("part", [NTT, 128, C], MBF16, kind="ExternalOutput").ap()

    with tile.TileContext(nc) as tc, ExitStack() as ctx:
        per = ctx.enter_context(tc.tile_pool(name="persist", bufs=1))
        xT = [
            per.tile([128, BT], MBF16, tag=f"xT{i}", name=f"xT{i}") for i in range(NCO)
        ]
        wq = per.tile([128, NCO, 128], MBF16, tag="wq")
        wk = per.tile([128, NCO, 128], MBF16, tag="wk")
        wv = per.tile([128, NCO, 128], MBF16, tag="wv")
        bq = per.tile([128, 1], F32, tag="bq")
        bk = per.tile([128, 1], F32, tag="bk")
        bv = per.tile([128, 1], F32, tag="bv")
        wo = per.tile([128, C], MBF16, tag="wo")
        mk = per.tile([128, 128], MBF16, tag="mk")
        ident = per.tile([128, 128], MBF16, tag="ident")
        qT = per.tile([128, BT], MBF16, tag="qT")
        kT = per.tile([128, BT], MBF16, tag="kT")
        vT = per.tile([128, BT], MBF16, tag="vT")
        # v natural per head, ones-column at col 64 (sumexp rides the PV matmul)
        vA = per.tile([128, NKT * B, 128], MBF16, tag="vA")
        vB = per.tile([128, NKT * B, 128], MBF16, tag="vB")
        # normalized attn out, head-major, one tile per 512-window so the
        # out-projection of window g only depends on window g's writers
        aT = [
            per.tile([128, 512], MBF16, tag=f"aT{g}", name=f"aT{g}")
            for g in range(NPW)
        ]

        # --- input DMAs: everything issued up front, split round-robin over
        # 3 engine queues, in need-order: wq, x w0, wk, wv, x w1, mask,
        # biases, wo, then the remaining x window pairs.
        engs = [nc.sync, nc.scalar]
        _eq = [0]

        def q_dma(out, in_):
            engs[_eq[0] % len(engs)].dma_start(out=out, in_=in_)
            _eq[0] += 1

        def load_x_window(wi, width=512):
            for i in range(NCO):
                q_dma(
                    xT[i][:, wi * 512 : wi * 512 + width],
                    xT_d[i][:, wi * 512 : wi * 512 + width],
                )

        def load_w(w_sb, w_d):
            for i in range(NCO):
                q_dma(w_sb[:, i, :], w_d[:, i, :])

        # engine-local setup first: runs while the DMAs stream in
        make_identity(nc, ident[:])
        nc.vector.memset(vA[:, :, 64:65], 1.0)
        nc.vector.memset(vB[:, :, 64:65], 1.0)
        # dummy matmuls on zeroed tiles: no input deps, so they run while the
        # first DMAs land — fills the startup PE gap and warms the HAM clock
        wrm_a = per.tile([128, 128], MBF16, tag="wrm_a")
        wrm_b = per.tile([128, 512], MBF16, tag="wrm_b")
        nc.vector.memset(wrm_a[:], 0.0)
        nc.vector.memset(wrm_b[:], 0.0)

        load_w(wq, wq_d)
        q_dma(bq[:], bq_d)
        q_dma(bk[:], bk_d)
        q_dma(bv[:], bv_d)
        load_x_window(0)
        load_w(wk, wk_d)
        load_w(wv, wv_d)
        load_x_window(1)
        q_dma(mk[:], mk_d)
        for i in range(NCO):
            q_dma(wo[:, ts(i, 128)], wo_d[:, ts(i, 128)])
        for wi in (2, 4, 6):
            load_x_window(wi, width=1024)

        with (
            tc.tile_pool(name="pps", bufs=2, space="PSUM") as pps,
            tc.tile_pool(name="sps", bufs=2, space="PSUM") as sps,
            tc.tile_pool(name="pvp", bufs=2, space="PSUM") as pvp,
            tc.tile_pool(name="eap", bufs=12) as eap,
            tc.tile_pool(name="rp", bufs=4) as rp,
            tc.tile_pool(name="bp", bufs=4) as bp,
            tc.tile_pool(name="tbp", bufs=4) as tbp,
            tc.tile_pool(name="stp", bufs=4) as stp,
        ):
            def warm_burst(n):
                wrm_ps = pps.tile([128, 512], F32, tag="proj", name="wrm_ps")
                for i in range(n):
                    nc.tensor.matmul(
                        wrm_ps[:], wrm_a[:], wrm_b[:], start=True, stop=True
                    )

            warm_burst(8)

            # --- filler work: small closures the attention stage loop pumps
            # into the PE stream to fill exp-wait gaps with useful matmuls.

            def proj_pieces(wi):
                """Projection of window wi as a list of closures."""
                state = {}

                def half(w_sb, b_sb, dest, lo):
                    def go():
                        if lo == 0:
                            state[id(b_sb)] = pps.tile(
                                [128, 512], F32, tag="proj", name="ps"
                            )
                        ps = state[id(b_sb)]
                        for co in range(lo, lo + 4):
                            nc.tensor.matmul(
                                ps[:],
                                w_sb[:, co, :],
                                xT[co][:, ts(wi, 512)],
                                start=(co == 0),
                                stop=(co == NCO - 1),
                            )
                        if lo == 4:
                            nc.vector.tensor_scalar_add(
                                dest[:, ts(wi, 512)], ps[:], b_sb[:, 0:1]
                            )
                    return go

                def vtrans(j):
                    def go():
                        tp = pps.tile([128, 128], MBF16, tag="proj", name="tp")
                        nc.tensor.transpose(tp[:], vT[:, ts(j, 128)], ident[:])
                        nc.vector.tensor_copy(out=vA[:, j, 0:64], in_=tp[:, 0:64])
                        nc.vector.tensor_copy(out=vB[:, j, 0:64], in_=tp[:, 64:128])
                    return go

                work = []
                for w_sb, b_sb, dest in ((wq, bq, qT), (wk, bk, kT), (wv, bv, vT)):
                    work.append(half(w_sb, b_sb, dest, 0))
                    work.append(half(w_sb, b_sb, dest, 4))
                for j in range(4 * wi, 4 * wi + 4):
                    work.append(vtrans(j))
                return work

            def proj_window(wi):
                for piece in proj_pieces(wi):
                    piece()

            def outproj_one(g, tt, on_act=False, tail=False):
                # tail=True: the scores pool is free after the last attention,
                # so the final out-projections use it for deeper buffering and
                # split their evacuation across DVE and ACT in parallel
                a_sl = aT[g][:, ts(tt - 4 * g, 128)]
                if tail:
                    op = sps.tile([128, 2, 512], F32, tag="s", name="opt")
                    op0, op1 = op[:, 0, :], op[:, 1, :]
                else:
                    op0 = pps.tile([128, 512], F32, tag="proj", name="op0")
                    op1 = pps.tile([128, 512], F32, tag="proj", name="op1")
                nc.tensor.matmul(op0, a_sl, wo[:, 0:512], start=True, stop=True)
                nc.tensor.matmul(op1, a_sl, wo[:, 512:1024], start=True, stop=True)
                st = stp.tile([128, 2, 512], MBF16, tag="st")
                if tail:
                    nc.vector.tensor_copy(out=st[:, 0, :], in_=op0)
                    nc.scalar.activation(st[:, 1, :], op1, AF.Copy)
                elif on_act:
                    nc.scalar.activation(st[:, 0, :], op0, AF.Copy)
                    nc.scalar.activation(st[:, 1, :], op1, AF.Copy)
                else:
                    nc.vector.tensor_copy(out=st[:, 0, :], in_=op0)
                    nc.vector.tensor_copy(out=st[:, 1, :], in_=op1)
                nc.sync.dma_start(out=out_d[tt], in_=st.rearrange("p a b -> p (a b)"))

            def outproj_pieces(g):
                return [
                    (lambda tt=tt: outproj_one(g, tt))
                    for tt in range(4 * g, 4 * g + 4)
                ]

            def attention(b, w, work=None):
                """work: list of closures pumped into the PE stream between
                attention stages (proj / out-proj of other windows)."""
                work = list(work) if work else []
                qs = b * T + w * 512
                nk = 4 * (w + 1)
                nstages = nk // 2
                pva = pvp.tile([128, 512], F32, tag="pv", name="pva")
                pvb = pvp.tile([128, 512], F32, tag="pv", name="pvb")

                def emit_pv_head(jp, e, vh, pv, c0s):
                    j0 = 2 * jp
                    for jj, jloc in ((0, j0), (1, j0 + 1)):
                        c0 = c0s[jj]
                        nc.tensor.matmul(
                            pv[0:65, ds(c0, 512 - c0)],
                            vh[:, b * NKT + jloc, 0:65],
                            e[:, jj, ds(c0, 512 - c0)],
                            start=(jloc == 0),
                            stop=(jloc == nk - 1),
                        )

                def exp_head(s_ps, jlocs, c0s, diag):
                    e = eap.tile([128, 2, 512], MBF16, tag="e")
                    if not diag:
                        nc.scalar.activation(e[:], s_ps[:], AF.Exp, scale=float(SCALE))
                    else:
                        # only the 128-wide diagonal strip needs the mask;
                        # columns past the strip are fully valid
                        for jj, jloc in ((0, jlocs[0]), (1, jlocs[1])):
                            c0 = c0s[jj]
                            cw = ds(c0, 512 - c0)
                            nc.scalar.activation(
                                e[:, jj, cw], s_ps[:, jj, cw], AF.Exp,
                                scale=float(SCALE),
                            )
                            nc.vector.tensor_mul(
                                e[:, jj, ds(c0, 128)],
                                e[:, jj, ds(c0, 128)],
                                mk[:],
                            )
                    return e

                # The first PV of a window waits for the previous window's
                # normalize to read its PSUM bank (pvp rotation), a ~2.5us
                # chain; so PV rides two stages behind the scores at the
                # window start (pends), then one stage behind (steady).
                pends = []

                def flush_pv():
                    jp0, ea0, eb0, c0s0 = pends.pop(0)
                    emit_pv_head(jp0, ea0, vA, pva, c0s0)
                    emit_pv_head(jp0, eb0, vB, pvb, c0s0)

                for jp in range(nstages):
                    j0, j1 = 2 * jp, 2 * jp + 1
                    diag = j0 >= nk - 4
                    # diagonal block i: columns < 128*i are fully masked, so
                    # scores/PV only touch columns [128*i, 512)
                    c0s = [
                        max(0, (jloc - (nk - 4)) * 128) if diag else 0
                        for jloc in (j0, j1)
                    ]
                    sa = sps.tile([128, 2, 512], F32, tag="s", name="sa")
                    sb_ = sps.tile([128, 2, 512], F32, tag="s", name="sb")
                    # interleave heads A/B so consecutive MMs target disjoint
                    # PE row groups (auto 64x128 tiling) and overlap in the
                    # array
                    for jj, jloc in ((0, j0), (1, j1)):
                        kd = ds(b * T + jloc * 128, 128)
                        c0 = c0s[jj]
                        cw = ds(c0, 512 - c0)
                        qd = ds(qs + c0, 512 - c0)
                        nc.tensor.matmul(
                            sa[:, jj, cw], kT[0:64, kd], qT[0:64, qd],
                            start=True, stop=True,
                        )
                        nc.tensor.matmul(
                            sb_[:, jj, cw], kT[64:128, kd], qT[64:128, qd],
                            start=True, stop=True,
                        )
                    ea = exp_head(sa, (j0, j1), c0s, diag)
                    eb = exp_head(sb_, (j0, j1), c0s, diag)
                    pends.append((jp, ea, eb, c0s))
                    if len(pends) > 2 or (len(pends) > 1 and jp >= 2):
                        flush_pv()
                    # pump filler work: spread the remaining closures over the
                    # remaining stages (plus the tail slot) so the PE always
                    # has independent work while the next scores wait on exp
                    npump = len(work) // (nstages - jp + 1) if work else 0
                    for _ in range(npump):
                        work.pop(0)()
                while pends:
                    half = len(work) // 2
                    for piece in work[:half]:
                        piece()
                    work = work[half:]
                    flush_pv()
                for piece in work:
                    piece()
                return pva, pvb

            def normalize(b, w, pva, pvb, cols=None):
                # rows 0..63 head dims, row 64 sumexp
                g = NQW * b + w
                c0, cn = cols if cols else (0, 512)
                cs = ds(c0, cn)
                for (pv, hi) in ((pva, 0), (pvb, 1)):
                    # custom-DVE recip misreads PSUM on HW: copy to SBUF first
                    sm = rp.tile([1, 512], F32, tag="sm", name="sm")
                    nc.scalar.activation(sm[0:1, 0:cn], pv[64:65, cs], AF.Copy)
                    rc = rp.tile([1, 512], F32, tag="rc", name="rc")
                    nc.vector.reciprocal_approx_fast(out=rc[0:1, 0:cn], in_=sm[0:1, 0:cn])
                    bc = bp.tile([64, 512], F32, tag="bc", name="bc")
                    nc.gpsimd.partition_broadcast(bc[:, 0:cn], rc[0:1, 0:cn], channels=64)
                    if hi == 0:
                        nc.vector.tensor_mul(aT[g][0:64, cs], pv[0:64, cs], bc[:, 0:cn])
                    else:
                        tb = tbp.tile([64, 512], MBF16, tag="tb")
                        nc.vector.tensor_mul(tb[:, 0:cn], pv[0:64, cs], bc[:, 0:cn])
                        # head B lives on partitions 64..127 of aT; DVE can't
                        # cross partitions, so hop through an SBUF->SBUF DMA.
                        nc.sync.dma_start(out=aT[g][64:128, cs], in_=tb[:, 0:cn])

            def att(b, w, work=None):
                pva, pvb = attention(b, w, work=work)
                normalize(b, w, pva, pvb)

            # emission order: b0 pipelined per window; b1 projections ride as
            # filler work inside earlier attentions; the small (1,0) window
            # runs last so the end-of-kernel normalize/out-proj chain is short.
            # proj pieces (no pending deps) lead each work list; outproj
            # pieces trail because they wait on the previous window's
            # normalize chain — pumping them early stalls the PE FIFO at the
            # window boundary, and that idle trips the HAM clock-gate.
            proj_window(0)
            att(0, 0, work=proj_pieces(1))
            att(0, 1, work=proj_pieces(2) + outproj_pieces(0))
            att(0, 2, work=proj_pieces(3) + outproj_pieces(1))
            att(0, 3, work=proj_pieces(4) + proj_pieces(5) + outproj_pieces(2))
            att(1, 1, work=proj_pieces(6) + outproj_pieces(3))
            att(1, 2, work=proj_pieces(7) + outproj_pieces(NQW + 1))
            att(1, 3, work=outproj_pieces(NQW + 2))
            # last window: normalize/out-project in half-window chunks so the
            # final out-proj matmuls overlap the second half's normalize;
            # warm-burst fillers bridge the normalize chains so the HAM
            # clock-gate stays open through the tail
            pva, pvb = attention(1, 0, work=outproj_pieces(NQW + 3))
            glast = NQW
            normalize(1, 0, pva, pvb, cols=(0, 256))
            warm_burst(5)
            outproj_tiles = list(range(4 * glast, 4 * glast + 4))
            for tt in outproj_tiles[:2]:
                outproj_one(glast, tt, tail=True)
            normalize(1, 0, pva, pvb, cols=(256, 256))
            warm_burst(5)
            for tt in outproj_tiles[2:]:
                outproj_one(glast, tt, tail=True)

        if dbg:
            for name, t in (("qTd", qT), ("kTd", kT), ("vTd", vT)):
                d = nc.dram_tensor(name, [128, BT], MBF16, kind="ExternalOutput").ap()
                nc.sync.dma_start(out=d, in_=t[:])
            aTd = nc.dram_tensor("aTd", [128, BT], MBF16, kind="ExternalOutput").ap()
            for g in range(NPW):
                nc.sync.dma_start(out=aTd[:, ts(g, 512)], in_=aT[g][:])
            for name, t in (("vAd", vA), ("vBd", vB)):
                d = nc.dram_tensor(
                    name, [128, NKT * B, 65], MBF16, kind="ExternalOutput"
                ).ap()
                nc.sync.dma_start(out=d, in_=t[:, :, 0:65])

    nc.compile()
    return nc


_NC = None


def _get_nc():
    global _NC
    if _NC is None:
        _NC = build_nc()
    return _NC


def _make_in_maps(x, w_qkv, b_qkv, w_out):
    xT = np.ascontiguousarray(x.reshape(BT, C).T).astype(BF16).reshape(NCO, 128, BT)
    p = np.arange(128)[:, None]
    f = np.arange(128)[None, :]
    mask = (p <= f).astype(BF16)
    in_maps = []
    for i in range(NCORES):
        sl = slice(128 * i, 128 * i + 128)
        m = {
            "xT": xT,
            "wq": np.ascontiguousarray(
                w_qkv[:, sl].reshape(NCO, 128, 128).transpose(1, 0, 2)
            ).astype(BF16),
            "wk": np.ascontiguousarray(
                w_qkv[:, C + 128 * i : C + 128 * i + 128]
                .reshape(NCO, 128, 128)
                .transpose(1, 0, 2)
            ).astype(BF16),
            "wv": np.ascontiguousarray(
                w_qkv[:, 2 * C + 128 * i : 2 * C + 128 * i + 128]
                .reshape(NCO, 128, 128)
                .transpose(1, 0, 2)
            ).astype(BF16),
            "bq": b_qkv[sl].astype(np.float32).reshape(128, 1),
            "bk": b_qkv[C + 128 * i : C + 128 * i + 128].astype(np.float32).reshape(128, 1),
            "bv": b_qkv[2 * C + 128 * i : 2 * C + 128 * i + 128]
            .astype(np.float32)
            .reshape(128, 1),
            "wo": np.ascontiguousarray(w_out[sl, :]).astype(BF16),
            "mask": mask,
        }
        in_maps.append(m)
    return in_maps


def run(inputs, trace=False):
    """Returns (y, exec_time_ns_or_None)."""
    x = np.asarray(inputs["x"], dtype=np.float32)
    w_qkv = np.asarray(inputs["w_qkv"], dtype=np.float32)
    b_qkv = np.asarray(inputs["b_qkv"], dtype=np.float32)
    w_out = np.asarray(inputs["w_out"], dtype=np.float32)
    b_out = np.asarray(inputs["b_out"], dtype=np.float32)

    nc = _get_nc()
    in_maps = _make_in_maps(x, w_qkv, b_qkv, w_out)
    res = run_bass_kernel_spmd(nc, in_maps, list(range(NCORES)), trace=trace)
    part = np.zeros((NTT, 128, C), dtype=np.float32)
    for r in res.results:
        part += r["part"]
    y = part.reshape(BT, C) + b_out[None, :]
    return y.reshape(B, T, C).astype(np.float32), res.exec_time_ns


def kernel(**inputs):
    return run(inputs, trace=False)[0]
